# revision 12
# baseline (speedup 1.0000x reference)
"""Bahdanau additive attention (vectorized) on TRN2 — Bass/Tile kernel.

Problem: nn_AttentionLayer_11055245820581
  e[b,y,x] = softmax_x( sum_e V[e] * tanh(Ws[b,x,e] + Uh[b,y,e]) )
  c[b,y,:] = sum_x e[b,y,x] * enc[b,x,:]
with Ws = enc @ W_a, Uh = dec @ U_a.

Sharding: data-parallel over batch B=8 across the 8 NeuronCores (one
batch element per core). Each core computes its batch's full attention.

Per-core layout: the tanh-cube work (Ty*Tx*E = 16.7M elements) keeps the
E axis on partitions (2 chunks of 128).
  - DVE  : broadcast-add WsT[e,x] + UhT[e,y] via tensor_scalar_add
           (per-partition scalar, fp32 2x mode).
  - ACT  : one big in-place Tanh per (y-block, chunk) slab to amortize
           the per-instruction overhead; ACT is the bottleneck engine.
  - PE   : projection with the tanh slab as float32r stationary
           ([128e, 32y] slices) and V as the moving operand, so e' lands
           directly as [y(partition), x] columns in PSUM.
  - softmax: ACT Exp with accum_out (free-dim row sum) + DVE reciprocal
           + tensor_scalar_mul; context via PE-transposed alpha (fp32).
"""

import numpy as np
from contextlib import ExitStack

import concourse.bass as bass
import concourse.bacc as bacc
import concourse.tile as tile
from concourse import mybir
from concourse.bass_utils import run_bass_kernel_spmd

B, Tx, Ty, E, D = 8, 256, 256, 256, 256
P = 128
NCORES = 8
F32 = mybir.dt.float32
F16 = mybir.dt.float16
TANH = mybir.ActivationFunctionType.Tanh
EXP = mybir.ActivationFunctionType.Exp

G = 32           # y-block size of the main loop
NB = Ty // G     # 8 blocks
EC = E // P      # 2 e-chunks
XC = Tx // P     # 2 x-chunks
YC = Ty // P     # 2 y-halves
DC = D // P      # 2 d-chunks

_NC = None
LAST_RESULTS = None


def _build_body(tc, ctx, enc_d, dec_d, W_d, U_d, V_d, c_d, e_d):
    nc = tc.nc
    from concourse.masks import make_identity

    consts = ctx.enter_context(tc.tile_pool(name="consts", bufs=1))
    add_pool = ctx.enter_context(tc.tile_pool(name="adds", bufs=2))
    tanh_pool = ctx.enter_context(tc.tile_pool(name="tanhs", bufs=4))
    out_pool = ctx.enter_context(tc.tile_pool(name="outs", bufs=2))
    setup_psum = ctx.enter_context(tc.tile_pool(name="psetup", bufs=2, space="PSUM"))
    e_psum = ctx.enter_context(tc.tile_pool(name="pe", bufs=1, space="PSUM"))
    fin_psum = ctx.enter_context(tc.tile_pool(name="pfin", bufs=2, space="PSUM"))

    # ---- load inputs ----
    enc_sb = consts.tile([P, XC, E], F32)    # [x_in_chunk, (xc), e]
    dec_sb = consts.tile([P, YC, D], F32)
    W_sb = consts.tile([P, EC, E], F32)      # rows e_in
    U_sb = consts.tile([P, DC, E], F32)      # rows d
    V_sb = consts.tile([P, EC], F32)
    for i in range(XC):
        nc.sync.dma_start(out=enc_sb[:, i, :], in_=enc_d[i * P:(i + 1) * P, :])
    for i in range(YC):
        nc.sync.dma_start(out=dec_sb[:, i, :], in_=dec_d[i * P:(i + 1) * P, :])
    for i in range(EC):
        nc.sync.dma_start(out=W_sb[:, i, :], in_=W_d[i * P:(i + 1) * P, :])
    for i in range(DC):
        nc.sync.dma_start(out=U_sb[:, i, :], in_=U_d[i * P:(i + 1) * P, :])
    for i in range(EC):
        nc.sync.dma_start(out=V_sb[:, i:i + 1], in_=V_d[i * P:(i + 1) * P, :])

    ident = consts.tile([P, P], F32)
    make_identity(nc, ident)

    # fp16 copy of V for the projection matmul (tanh slab is fp16 too:
    # |tanh|<=1 so fp16's 2^-11 relative quantization is plenty).
    V16_sb = consts.tile([P, EC], F16)
    nc.vector.tensor_copy(V16_sb[:], V_sb[:])

    # ---- transpose enc, dec (PE transpose via identity) ----
    encT_sb = consts.tile([P, EC, Tx], F32)  # [e, (ec), x]
    decT_sb = consts.tile([P, DC, Ty], F32)  # [d, (dc), y]
    for src, srcC, dstT, dstC in ((enc_sb, XC, encT_sb, EC),
                                  (dec_sb, YC, decT_sb, DC)):
        for i in range(srcC):          # source partition chunk (x or y)
            for j in range(dstC):      # source free chunk (e or d)
                pt = setup_psum.tile([P, Tx], F32, tag="ps", name="pt")
                nc.tensor.transpose(
                    out=pt[:, :P], in_=src[:, i, j * P:(j + 1) * P],
                    identity=ident[:])
                nc.vector.tensor_copy(dstT[:, j, i * P:(i + 1) * P], pt[:, :P])

    # ---- WsT[e_out, x] = sum_ei W[ei, e_out] * encT[ei, x]  (fp32 exact) ----
    WsT_sb = consts.tile([P, EC, Tx], F32)
    UhT_sb = consts.tile([P, EC, Ty], F32)
    for co in range(EC):
        pw = setup_psum.tile([P, Tx], F32, tag="ps", name="pw")
        for ci in range(EC):
            nc.tensor.matmul(
                out=pw[:], lhsT=W_sb[:, ci, co * P:(co + 1) * P],
                rhs=encT_sb[:, ci, :], start=(ci == 0), stop=(ci == EC - 1))
        nc.vector.tensor_copy(WsT_sb[:, co, :], pw[:])
    for co in range(EC):
        pu = setup_psum.tile([P, Ty], F32, tag="ps", name="pu")
        for ci in range(DC):
            nc.tensor.matmul(
                out=pu[:], lhsT=U_sb[:, ci, co * P:(co + 1) * P],
                rhs=decT_sb[:, ci, :], start=(ci == 0), stop=(ci == DC - 1))
        nc.vector.tensor_copy(UhT_sb[:, co, :], pu[:])

    # ---- main loop: tanh cube + V projection ----
    # e'[y, x] accumulates into two [128, Tx] PSUM tiles (one per y-half).
    e_ps = [e_psum.tile([P, Tx], F32, tag=f"e{h}", name=f"e_ps{h}")
            for h in range(YC)]
    # Zero-init via DVE so every projection matmul can be a pure
    # accumulate (start=False): no reliance on PSUM zero-region marking,
    # whose whole-bank granularity breaks per-column start/stop groups.
    for h in range(YC):
        nc.vector.memset(e_ps[h][:], 0.0)

    for b in range(NB):
        y0 = b * G
        h = y0 // P
        p0 = y0 % P
        slabs = []
        for c in range(EC):
            aslab = add_pool.tile([P, G, Tx], F32, tag="add", name=f"add{b}_{c}")
            for j in range(G):
                nc.vector.tensor_scalar_add(
                    out=aslab[:, j, :], in0=WsT_sb[:, c, :],
                    scalar1=UhT_sb[:, c, y0 + j:y0 + j + 1])
            tslab = tanh_pool.tile([P, G, Tx], F16, tag="tanh",
                                   name=f"tanh{b}_{c}")
            nc.scalar.activation(out=tslab[:], in_=aslab[:], func=TANH)
            slabs.append(tslab)
        for c in range(EC):
            for x in range(Tx):
                nc.tensor.matmul(
                    out=e_ps[h][p0:p0 + G, x:x + 1],
                    lhsT=slabs[c][:, :, x],
                    rhs=V16_sb[:, c:c + 1],
                    start=False, stop=False,
                    skip_group_check=True,
                    tile_position=(0, p0))

    # ---- softmax over x (free dim) ----
    alpha_sb = consts.tile([P, YC, Tx], F32)
    for h in range(YC):
        ex = out_pool.tile([P, Tx], F32, tag="ex", name=f"ex{h}")
        denom = out_pool.tile([P, 1], F32, tag="denom", name=f"denom{h}")
        nc.scalar.activation(out=ex[:], in_=e_ps[h][:], func=EXP,
                             accum_out=denom[:])
        recip = out_pool.tile([P, 1], F32, tag="recip", name=f"recip{h}")
        nc.vector.reciprocal(recip[:], denom[:])
        nc.vector.tensor_scalar_mul(
            out=alpha_sb[:, h, :], in0=ex[:], scalar1=recip[:])
        nc.sync.dma_start(out=e_d[h * P:(h + 1) * P, :], in_=alpha_sb[:, h, :])

    # ---- alpha^T then context c = alpha @ enc (fp32 exact) ----
    alphaT_sb = consts.tile([P, XC, Ty], F32)  # [x, (xc), y]
    for h in range(YC):
        for xc in range(XC):
            pt2 = fin_psum.tile([P, E], F32, tag="pf", name="pt2")
            nc.tensor.transpose(
                out=pt2[:, :P], in_=alpha_sb[:, h, xc * P:(xc + 1) * P],
                identity=ident[:])
            nc.vector.tensor_copy(alphaT_sb[:, xc, h * P:(h + 1) * P],
                                  pt2[:, :P])
    for h in range(YC):
        pc = fin_psum.tile([P, E], F32, tag="pf", name=f"pc{h}")
        for xc in range(XC):
            nc.tensor.matmul(
                out=pc[:], lhsT=alphaT_sb[:, xc, h * P:(h + 1) * P],
                rhs=enc_sb[:, xc, :], start=(xc == 0), stop=(xc == XC - 1))
        c_sb = out_pool.tile([P, E], F32, tag="c_sb", name=f"c_sb{h}")
        nc.vector.tensor_copy(c_sb[:], pc[:])
        nc.sync.dma_start(out=c_d[h * P:(h + 1) * P, :], in_=c_sb[:])


def _build():
    nc = bacc.Bacc("TRN2", target_bir_lowering=False, debug=False,
                   num_devices=NCORES)
    enc_d = nc.dram_tensor("enc", [Tx, E], F32, kind="ExternalInput").ap()
    dec_d = nc.dram_tensor("dec", [Ty, D], F32, kind="ExternalInput").ap()
    W_d = nc.dram_tensor("W", [E, E], F32, kind="ExternalInput").ap()
    U_d = nc.dram_tensor("U", [D, E], F32, kind="ExternalInput").ap()
    V_d = nc.dram_tensor("V", [E, 1], F32, kind="ExternalInput").ap()
    c_d = nc.dram_tensor("c_out", [Ty, E], F32, kind="ExternalOutput").ap()
    e_d = nc.dram_tensor("e_out", [Ty, Tx], F32, kind="ExternalOutput").ap()

    with tile.TileContext(nc) as tc:
        with ExitStack() as ctx:
            _build_body(tc, ctx, enc_d, dec_d, W_d, U_d, V_d, c_d, e_d)
    nc.compile()
    return nc


def _get_nc():
    global _NC
    if _NC is None:
        _NC = _build()
    return _NC


def kernel(encoder_out_seq, decoder_out_seq, W_a, U_a, V_a):
    enc = np.ascontiguousarray(np.asarray(encoder_out_seq, dtype=np.float32))
    dec = np.ascontiguousarray(np.asarray(decoder_out_seq, dtype=np.float32))
    W = np.ascontiguousarray(np.asarray(W_a, dtype=np.float32))
    U = np.ascontiguousarray(np.asarray(U_a, dtype=np.float32))
    V = np.ascontiguousarray(np.asarray(V_a, dtype=np.float32))

    nc = _get_nc()
    in_maps = [
        {"enc": enc[i], "dec": dec[i], "W": W, "U": U, "V": V}
        for i in range(NCORES)
    ]
    res = run_bass_kernel_spmd(nc, in_maps, list(range(NCORES)))
    global LAST_RESULTS
    LAST_RESULTS = res
    c = np.stack([res.results[i]["c_out"] for i in range(NCORES)])
    e = np.stack([res.results[i]["e_out"] for i in range(NCORES)])
    return c, e


# revision 13
# speedup vs baseline: 1.2499x; 1.2499x over previous
"""Bahdanau additive attention (vectorized) on TRN2 — Bass/Tile kernel.

Problem: nn_AttentionLayer_11055245820581
  e[b,y,x] = softmax_x( sum_e V[e] * tanh(Ws[b,x,e] + Uh[b,y,e]) )
  c[b,y,:] = sum_x e[b,y,x] * enc[b,x,:]
with Ws = enc @ W_a, Uh = dec @ U_a.

Sharding: data-parallel over batch B=8 across the 8 NeuronCores (one
batch element per core). Each core computes its batch's full attention.

Per-core dataflow (the tanh cube Ty*Tx*E = 16.7M elements dominates;
ACT's 1 elem/lane/cycle tanh is the ~110us floor, everything else is
arranged to stay below it):
  - broadcast-add WsT[e,x] + UhT[e,y] into fp16 slabs, split per block
    between DVE (tensor_scalar_add, per-partition scalar) and GPSIMD
    (one tensor_tensor with step-0 broadcast APs per 32-y slab) so
    neither engine exceeds ACT's per-block budget.
  - ACT: one big Tanh per (y-block, chunk) slab -> fp16 tanh slab.
  - PE: projection with tanh slab as fp16 stationary [128e, 128x] and
    V fp16 moving: e'^T lands as [x(partition), y] columns in PSUM
    (M=128 amortizes the per-matmul fixed cost; no PSUM evacuation).
  - softmax in the transposed layout: ACT Exp -> expT in SBUF; row sums
    over x via matmul with a ones vector -> denom[y]; DVE reciprocal;
    context matmul uses unnormalized expT and scales c rows by 1/denom;
    attention weights are PE-transposed back to [y, x] and scaled.
"""

import numpy as np
from contextlib import ExitStack

import concourse.bass as bass
import concourse.bacc as bacc
import concourse.tile as tile
from concourse import mybir
from concourse.bass_utils import run_bass_kernel_spmd

B, Tx, Ty, E, D = 8, 256, 256, 256, 256
P = 128
NCORES = 8
F32 = mybir.dt.float32
F16 = mybir.dt.float16
TANH = mybir.ActivationFunctionType.Tanh
EXP = mybir.ActivationFunctionType.Exp

G = 32           # y-block size of the main loop
NB = Ty // G     # 8 blocks
EC = E // P      # 2 e-chunks
XC = Tx // P     # 2 x-chunks
YC = Ty // P     # 2 y-halves
DC = D // P      # 2 d-chunks

_NC = None
LAST_RESULTS = None


def _bcast_add_ap(t, n_rep, n_inner):
    """AP reading a [P, n_inner] tile as [P, n_rep, n_inner] (repeat dim 1)."""
    return bass.AP(tensor=t.tensor, offset=t.offset,
                   ap=[t.ap[0], [0, n_rep], t.ap[1]])


def _bcast_inner_ap(t, col0, n_rep, n_inner):
    """AP reading tile columns [col0:col0+n_rep] as [P, n_rep, n_inner]
    (each column repeated n_inner times along the innermost dim)."""
    step = t.ap[1][0]
    return bass.AP(tensor=t.tensor, offset=t.offset + col0 * step,
                   ap=[t.ap[0], [step, n_rep], [0, n_inner]])


def _build_body(tc, ctx, enc_d, dec_d, W_d, U_d, V_d, c_d, e_d):
    nc = tc.nc
    from concourse.masks import make_identity

    consts = ctx.enter_context(tc.tile_pool(name="consts", bufs=1))
    add_pool = ctx.enter_context(tc.tile_pool(name="adds", bufs=4))
    tanh_pool = ctx.enter_context(tc.tile_pool(name="tanhs", bufs=4))
    out_pool = ctx.enter_context(tc.tile_pool(name="outs", bufs=2))
    setup_psum = ctx.enter_context(tc.tile_pool(name="psetup", bufs=2, space="PSUM"))
    e_psum = ctx.enter_context(tc.tile_pool(name="pe", bufs=1, space="PSUM"))
    fin_psum = ctx.enter_context(tc.tile_pool(name="pfin", bufs=2, space="PSUM"))

    # ---- load inputs ----
    enc_sb = consts.tile([P, XC, E], F32)    # [x_in_chunk, (xc), e]
    dec_sb = consts.tile([P, YC, D], F32)
    W_sb = consts.tile([P, EC, E], F32)      # rows e_in
    U_sb = consts.tile([P, DC, E], F32)      # rows d
    V_sb = consts.tile([P, EC], F32)
    for i in range(XC):
        nc.sync.dma_start(out=enc_sb[:, i, :], in_=enc_d[i * P:(i + 1) * P, :])
    for i in range(YC):
        nc.sync.dma_start(out=dec_sb[:, i, :], in_=dec_d[i * P:(i + 1) * P, :])
    for i in range(EC):
        nc.sync.dma_start(out=W_sb[:, i, :], in_=W_d[i * P:(i + 1) * P, :])
    for i in range(DC):
        nc.sync.dma_start(out=U_sb[:, i, :], in_=U_d[i * P:(i + 1) * P, :])
    for i in range(EC):
        nc.sync.dma_start(out=V_sb[:, i:i + 1], in_=V_d[i * P:(i + 1) * P, :])

    ident = consts.tile([P, P], F32)
    make_identity(nc, ident)
    ones_sb = consts.tile([P, 1], F32)
    nc.vector.memset(ones_sb[:], 1.0)
    V16_sb = consts.tile([P, EC], F16)
    nc.vector.tensor_copy(V16_sb[:], V_sb[:])

    # ---- transpose enc, dec (PE transpose via identity) ----
    encT_sb = consts.tile([P, EC, Tx], F32)  # [e, (ec), x]
    decT_sb = consts.tile([P, DC, Ty], F32)  # [d, (dc), y]
    for src, srcC, dstT, dstC in ((enc_sb, XC, encT_sb, EC),
                                  (dec_sb, YC, decT_sb, DC)):
        for i in range(srcC):          # source partition chunk (x or y)
            for j in range(dstC):      # source free chunk (e or d)
                pt = setup_psum.tile([P, Tx], F32, tag="ps", name="pt")
                nc.tensor.transpose(
                    out=pt[:, :P], in_=src[:, i, j * P:(j + 1) * P],
                    identity=ident[:])
                nc.vector.tensor_copy(dstT[:, j, i * P:(i + 1) * P], pt[:, :P])

    # ---- WsT[e_out, x] = sum_ei W[ei, e_out] * encT[ei, x] ----
    # fp16 copies feed the DVE/GPSIMD adds; fp32 UhT feeds the DVE
    # per-partition scalar reads (TensorScalar requires fp32 scalars).
    WsT16_sb = consts.tile([P, EC, Tx], F16)
    UhT16_sb = consts.tile([P, EC, Ty], F16)
    UhT_sb = consts.tile([P, EC, Ty], F32)
    for co in range(EC):
        pw = setup_psum.tile([P, Tx], F32, tag="ps", name="pw")
        for ci in range(EC):
            nc.tensor.matmul(
                out=pw[:], lhsT=W_sb[:, ci, co * P:(co + 1) * P],
                rhs=encT_sb[:, ci, :], start=(ci == 0), stop=(ci == EC - 1))
        nc.vector.tensor_copy(WsT16_sb[:, co, :], pw[:])
    for co in range(EC):
        pu = setup_psum.tile([P, Ty], F32, tag="ps", name="pu")
        for ci in range(DC):
            nc.tensor.matmul(
                out=pu[:], lhsT=U_sb[:, ci, co * P:(co + 1) * P],
                rhs=decT_sb[:, ci, :], start=(ci == 0), stop=(ci == DC - 1))
        nc.vector.tensor_copy(UhT_sb[:, co, :], pu[:])
        nc.vector.tensor_copy(UhT16_sb[:, co, :], pu[:])

    # ---- main loop: tanh cube + V projection into e'^T ----
    # e'^T[x, y] accumulates into two [128, Ty] PSUM tiles (one per xc).
    eT_ps = [e_psum.tile([P, Ty], F32, tag=f"e{i}", name=f"eT_ps{i}")
             for i in range(XC)]
    for i in range(XC):
        nc.vector.memset(eT_ps[i][:], 0.0)

    for b in range(NB):
        y0 = b * G
        slabs = []
        for c in range(EC):
            aslab = add_pool.tile([P, G, Tx], F16, tag="add", name=f"add{b}_{c}")
            if c == 1:
                # GPSIMD: one broadcast tensor_tensor for the whole slab.
                nc.gpsimd.tensor_add(
                    aslab[:],
                    _bcast_add_ap(WsT16_sb[:, c, :], G, Tx),
                    _bcast_inner_ap(UhT16_sb[:, c, :], y0, G, Tx))
            else:
                # DVE: per-y tensor_scalar with per-partition fp32 scalar.
                for j in range(G):
                    nc.vector.tensor_scalar_add(
                        out=aslab[:, j, :], in0=WsT16_sb[:, c, :],
                        scalar1=UhT_sb[:, c, y0 + j:y0 + j + 1])
            tslab = tanh_pool.tile([P, G, Tx], F16, tag="tanh",
                                   name=f"tanh{b}_{c}")
            nc.scalar.activation(out=tslab[:], in_=aslab[:], func=TANH)
            slabs.append(tslab)
        for j in range(G):
            for xc in range(XC):
                for c in range(EC):
                    nc.tensor.matmul(
                        out=eT_ps[xc][:, y0 + j:y0 + j + 1],
                        lhsT=slabs[c][:, j, xc * P:(xc + 1) * P],
                        rhs=V16_sb[:, c:c + 1],
                        start=False, stop=False,
                        skip_group_check=True)

    # ---- softmax over x (partition dim of e'^T) ----
    expT_sb = consts.tile([P, XC, Ty], F32)  # [x, (xc), y]
    for xc in range(XC):
        nc.scalar.activation(out=expT_sb[:, xc, :], in_=eT_ps[xc][:], func=EXP)
    den_ps = fin_psum.tile([P, YC], F32, tag="den", name="den_ps")
    nc.vector.memset(den_ps[:], 0.0)
    for yh in range(YC):
        for xc in range(XC):
            nc.tensor.matmul(
                out=den_ps[:, yh:yh + 1],
                lhsT=expT_sb[:, xc, yh * P:(yh + 1) * P],
                rhs=ones_sb[:],
                start=False, stop=False, skip_group_check=True)
    recip_sb = consts.tile([P, YC], F32)
    nc.vector.reciprocal(recip_sb[:], den_ps[:])

    # ---- context c[y,:] = (sum_x expT[x,y] * enc[x,:]) / denom[y] ----
    for yh in range(YC):
        pc = fin_psum.tile([P, E], F32, tag="pf", name=f"pc{yh}")
        for xc in range(XC):
            nc.tensor.matmul(
                out=pc[:], lhsT=expT_sb[:, xc, yh * P:(yh + 1) * P],
                rhs=enc_sb[:, xc, :], start=(xc == 0), stop=(xc == XC - 1))
        c_sb = out_pool.tile([P, E], F32, tag="c_sb", name=f"c_sb{yh}")
        nc.vector.tensor_scalar_mul(
            out=c_sb[:], in0=pc[:], scalar1=recip_sb[:, yh:yh + 1])
        nc.sync.dma_start(out=c_d[yh * P:(yh + 1) * P, :], in_=c_sb[:])

    # ---- attention weights alpha[y,x] = expT[x,y]^T / denom[y] ----
    alpha_sb = consts.tile([P, YC, Tx], F32)
    for yh in range(YC):
        for xc in range(XC):
            pt2 = fin_psum.tile([P, E], F32, tag="pf", name="pt2")
            nc.tensor.transpose(
                out=pt2[:, :P], in_=expT_sb[:, xc, yh * P:(yh + 1) * P],
                identity=ident[:])
            nc.vector.tensor_scalar_mul(
                out=alpha_sb[:, yh, xc * P:(xc + 1) * P], in0=pt2[:, :P],
                scalar1=recip_sb[:, yh:yh + 1])
        nc.sync.dma_start(out=e_d[yh * P:(yh + 1) * P, :],
                          in_=alpha_sb[:, yh, :])


def _build():
    nc = bacc.Bacc("TRN2", target_bir_lowering=False, debug=False,
                   num_devices=NCORES)
    enc_d = nc.dram_tensor("enc", [Tx, E], F32, kind="ExternalInput").ap()
    dec_d = nc.dram_tensor("dec", [Ty, D], F32, kind="ExternalInput").ap()
    W_d = nc.dram_tensor("W", [E, E], F32, kind="ExternalInput").ap()
    U_d = nc.dram_tensor("U", [D, E], F32, kind="ExternalInput").ap()
    V_d = nc.dram_tensor("V", [E, 1], F32, kind="ExternalInput").ap()
    c_d = nc.dram_tensor("c_out", [Ty, E], F32, kind="ExternalOutput").ap()
    e_d = nc.dram_tensor("e_out", [Ty, Tx], F32, kind="ExternalOutput").ap()

    with tile.TileContext(nc) as tc:
        with ExitStack() as ctx:
            _build_body(tc, ctx, enc_d, dec_d, W_d, U_d, V_d, c_d, e_d)
    nc.compile()
    return nc


def _get_nc():
    global _NC
    if _NC is None:
        _NC = _build()
    return _NC


def kernel(encoder_out_seq, decoder_out_seq, W_a, U_a, V_a):
    enc = np.ascontiguousarray(np.asarray(encoder_out_seq, dtype=np.float32))
    dec = np.ascontiguousarray(np.asarray(decoder_out_seq, dtype=np.float32))
    W = np.ascontiguousarray(np.asarray(W_a, dtype=np.float32))
    U = np.ascontiguousarray(np.asarray(U_a, dtype=np.float32))
    V = np.ascontiguousarray(np.asarray(V_a, dtype=np.float32))

    nc = _get_nc()
    in_maps = [
        {"enc": enc[i], "dec": dec[i], "W": W, "U": U, "V": V}
        for i in range(NCORES)
    ]
    res = run_bass_kernel_spmd(nc, in_maps, list(range(NCORES)))
    global LAST_RESULTS
    LAST_RESULTS = res
    c = np.stack([res.results[i]["c_out"] for i in range(NCORES)])
    e = np.stack([res.results[i]["e_out"] for i in range(NCORES)])
    return c, e


# revision 17
# speedup vs baseline: 1.6929x; 1.3544x over previous
"""Bahdanau additive attention (vectorized) on TRN2 — Bass/Tile kernel.

Problem: nn_AttentionLayer_11055245820581
  e[b,y,x] = softmax_x( sum_e V[e] * tanh(Ws[b,x,e] + Uh[b,y,e]) )
  c[b,y,:] = sum_x e[b,y,x] * enc[b,x,:]
with Ws = enc @ W_a, Uh = dec @ U_a.

Sharding: data-parallel over batch B=8 across the 8 NeuronCores (one
batch element per core). Each core computes its batch's full attention.

Per-core dataflow (the tanh cube Ty*Tx*E = 16.7M elements dominates;
ACT's 1 elem/lane/cycle tanh is the ~110us floor, everything else is
arranged to stay below it):
  - broadcast-add WsT[e,x] + UhT[e,y] into fp16 slabs, split per block
    between DVE (tensor_scalar_add, per-partition scalar) and GPSIMD
    (one tensor_tensor with step-0 broadcast APs per 32-y slab) so
    neither engine exceeds ACT's per-block budget.
  - ACT: one big Tanh per (y-block, chunk) slab -> fp16 tanh slab.
  - PE: projection with tanh slab as fp16 stationary [128e, 128x] and
    V fp16 moving: e'^T lands as [x(partition), y] columns in PSUM
    (M=128 amortizes the per-matmul fixed cost; no PSUM evacuation).
  - softmax in the transposed layout: ACT Exp -> expT in SBUF; row sums
    over x via matmul with a ones vector -> denom[y]; DVE reciprocal;
    context matmul uses unnormalized expT and scales c rows by 1/denom;
    attention weights are PE-transposed back to [y, x] and scaled.
"""

import numpy as np
from contextlib import ExitStack

import concourse.bass as bass
import concourse.bacc as bacc
import concourse.tile as tile
from concourse import mybir
from concourse.bass_utils import run_bass_kernel_spmd

B, Tx, Ty, E, D = 8, 256, 256, 256, 256
P = 128
NCORES = 8
F32 = mybir.dt.float32
F16 = mybir.dt.float16
TANH = mybir.ActivationFunctionType.Tanh
EXP = mybir.ActivationFunctionType.Exp

G = 32           # y-block size of the main loop
NB = Ty // G     # 8 blocks
EC = E // P      # 2 e-chunks
XC = Tx // P     # 2 x-chunks
YC = Ty // P     # 2 y-halves
DC = D // P      # 2 d-chunks

_NC = None
LAST_RESULTS = None


def _bcast_add_ap(t, n_rep, n_inner):
    """AP reading a [P, n_inner] tile as [P, n_rep, n_inner] (repeat dim 1)."""
    return bass.AP(tensor=t.tensor, offset=t.offset,
                   ap=[t.ap[0], [0, n_rep], t.ap[1]])


def _bcast_inner_ap(t, col0, n_rep, n_inner):
    """AP reading tile columns [col0:col0+n_rep] as [P, n_rep, n_inner]
    (each column repeated n_inner times along the innermost dim)."""
    step = t.ap[1][0]
    return bass.AP(tensor=t.tensor, offset=t.offset + col0 * step,
                   ap=[t.ap[0], [step, n_rep], [0, n_inner]])


def _build_body(tc, ctx, enc_d, dec_d, W_d, U_d, V_d, c_d, e_d):
    nc = tc.nc
    from concourse.masks import make_identity

    consts = ctx.enter_context(tc.tile_pool(name="consts", bufs=1))
    add_pool = ctx.enter_context(tc.tile_pool(name="adds", bufs=4))
    tanh_pool = ctx.enter_context(tc.tile_pool(name="tanhs", bufs=4))
    out_pool = ctx.enter_context(tc.tile_pool(name="outs", bufs=2))
    setup_psum = ctx.enter_context(tc.tile_pool(name="psetup", bufs=1, space="PSUM"))
    e_psum = ctx.enter_context(tc.tile_pool(name="pe", bufs=1, space="PSUM"))
    fin_psum = ctx.enter_context(tc.tile_pool(name="pfin", bufs=2, space="PSUM"))
    den_psum = ctx.enter_context(tc.tile_pool(name="pden", bufs=1, space="PSUM"))
    piece_psum = ctx.enter_context(tc.tile_pool(name="ppiece", bufs=2, space="PSUM"))

    # ---- load inputs ----
    enc_sb = consts.tile([P, XC, E], F32)    # [x_in_chunk, (xc), e]
    dec_sb = consts.tile([P, YC, D], F32)
    W_sb = consts.tile([P, EC, E], F32)      # rows e_in
    U_sb = consts.tile([P, DC, E], F32)      # rows d
    V_sb = consts.tile([P, EC], F32)
    for i in range(XC):
        nc.sync.dma_start(out=enc_sb[:, i, :], in_=enc_d[i * P:(i + 1) * P, :])
    for i in range(YC):
        nc.sync.dma_start(out=dec_sb[:, i, :], in_=dec_d[i * P:(i + 1) * P, :])
    for i in range(EC):
        nc.sync.dma_start(out=W_sb[:, i, :], in_=W_d[i * P:(i + 1) * P, :])
    for i in range(DC):
        nc.sync.dma_start(out=U_sb[:, i, :], in_=U_d[i * P:(i + 1) * P, :])
    for i in range(EC):
        nc.sync.dma_start(out=V_sb[:, i:i + 1], in_=V_d[i * P:(i + 1) * P, :])

    ident = consts.tile([P, P], F32)
    make_identity(nc, ident)
    ident16 = consts.tile([P, P], F16)
    nc.vector.tensor_copy(ident16[:], ident[:])
    ones_sb = consts.tile([P, 1], F32)
    nc.vector.memset(ones_sb[:], 1.0)
    V16_sb = consts.tile([P, EC], F16)
    nc.vector.tensor_copy(V16_sb[:], V_sb[:])

    # ---- transpose enc, dec (PE transpose via identity) ----
    encT_sb = consts.tile([P, EC, Tx], F32)  # [e, (ec), x]
    decT_sb = consts.tile([P, DC, Ty], F32)  # [d, (dc), y]
    for src, srcC, dstT, dstC in ((enc_sb, XC, encT_sb, EC),
                                  (dec_sb, YC, decT_sb, DC)):
        for i in range(srcC):          # source partition chunk (x or y)
            for j in range(dstC):      # source free chunk (e or d)
                pt = setup_psum.tile([P, Tx], F32, tag="ps", name="pt")
                nc.tensor.transpose(
                    out=pt[:, :P], in_=src[:, i, j * P:(j + 1) * P],
                    identity=ident[:])
                nc.vector.tensor_copy(dstT[:, j, i * P:(i + 1) * P], pt[:, :P])

    # ---- WsT[e_out, x] = sum_ei W[ei, e_out] * encT[ei, x] ----
    # fp16 copies feed the DVE/GPSIMD adds; fp32 UhT feeds the DVE
    # per-partition scalar reads (TensorScalar requires fp32 scalars).
    WsT16_sb = consts.tile([P, EC, Tx], F16)
    UhT16_sb = consts.tile([P, EC, Ty], F16)
    UhT_sb = consts.tile([P, EC, Ty], F32)
    for co in range(EC):
        pw = setup_psum.tile([P, Tx], F32, tag="ps", name="pw")
        for ci in range(EC):
            nc.tensor.matmul(
                out=pw[:], lhsT=W_sb[:, ci, co * P:(co + 1) * P],
                rhs=encT_sb[:, ci, :], start=(ci == 0), stop=(ci == EC - 1))
        nc.vector.tensor_copy(WsT16_sb[:, co, :], pw[:])
    for co in range(EC):
        pu = setup_psum.tile([P, Ty], F32, tag="ps", name="pu")
        for ci in range(DC):
            nc.tensor.matmul(
                out=pu[:], lhsT=U_sb[:, ci, co * P:(co + 1) * P],
                rhs=decT_sb[:, ci, :], start=(ci == 0), stop=(ci == DC - 1))
        nc.vector.tensor_copy(UhT_sb[:, co, :], pu[:])
        nc.vector.tensor_copy(UhT16_sb[:, co, :], pu[:])

    # ---- main loop: tanh cube + V projection into e'^T ----
    # e'^T[x, y] accumulates into two [128, Ty] PSUM tiles (one per xc).
    eT_ps = [e_psum.tile([P, Ty], F32, tag=f"e{i}", name=f"eT_ps{i}")
             for i in range(XC)]
    for i in range(XC):
        nc.vector.memset(eT_ps[i][:], 0.0)

    # First PY y's of every (block, chunk) get their broadcast-add done on
    # the Tensor engine (identity matmul of a step-0-broadcast W plus an
    # inner-broadcast U, accumulated in a PSUM piece), the rest on DVE via
    # fp16 tensor_scalar. ACT tanh-reads the PSUM pieces directly.
    PY = 4
    for b in range(NB):
        y0 = b * G
        slabs = []
        for c in range(EC):
            tslab = tanh_pool.tile([P, G, Tx], F16, tag="tanh",
                                   name=f"tanh{b}_{c}")
            for pj in range(PY // 2):
                piece = piece_psum.tile([P, 2 * Tx], F32, tag="piece",
                                        name=f"piece{b}_{c}_{pj}")
                nc.tensor.matmul(
                    out=piece[:],
                    lhsT=ident16[:],
                    rhs=_bcast_add_ap(WsT16_sb[:, c, :], 2, Tx),
                    start=True, stop=False)
                nc.tensor.matmul(
                    out=piece[:],
                    lhsT=ident16[:],
                    rhs=_bcast_inner_ap(UhT16_sb[:, c, :], y0 + 2 * pj, 2, Tx),
                    start=False, stop=True)
                nc.scalar.activation(
                    out=tslab[:, 2 * pj:2 * pj + 2, :], in_=piece[:],
                    func=TANH)
            aslab = add_pool.tile([P, G - PY, Tx], F16, tag="add",
                                  name=f"add{b}_{c}")
            for j in range(G - PY):
                nc.vector.tensor_scalar_add(
                    out=aslab[:, j, :], in0=WsT16_sb[:, c, :],
                    scalar1=UhT_sb[:, c, y0 + PY + j:y0 + PY + j + 1])
            nc.scalar.activation(out=tslab[:, PY:, :], in_=aslab[:],
                                 func=TANH)
            slabs.append(tslab)
        for j in range(G):
            for xc in range(XC):
                for c in range(EC):
                    nc.tensor.matmul(
                        out=eT_ps[xc][:, y0 + j:y0 + j + 1],
                        lhsT=slabs[c][:, j, xc * P:(xc + 1) * P],
                        rhs=V16_sb[:, c:c + 1],
                        start=False, stop=False,
                        skip_group_check=True)

    # ---- softmax over x (partition dim of e'^T) ----
    expT_sb = consts.tile([P, XC, Ty], F32)  # [x, (xc), y]
    for xc in range(XC):
        nc.scalar.activation(out=expT_sb[:, xc, :], in_=eT_ps[xc][:], func=EXP)
    den_ps = den_psum.tile([P, YC], F32, tag="den", name="den_ps")
    nc.vector.memset(den_ps[:], 0.0)
    for yh in range(YC):
        for xc in range(XC):
            nc.tensor.matmul(
                out=den_ps[:, yh:yh + 1],
                lhsT=expT_sb[:, xc, yh * P:(yh + 1) * P],
                rhs=ones_sb[:],
                start=False, stop=False, skip_group_check=True)
    recip_sb = consts.tile([P, YC], F32)
    nc.vector.reciprocal(recip_sb[:], den_ps[:])

    # ---- context c[y,:] = (sum_x expT[x,y] * enc[x,:]) / denom[y] ----
    for yh in range(YC):
        pc = fin_psum.tile([P, E], F32, tag="pf", name=f"pc{yh}")
        for xc in range(XC):
            nc.tensor.matmul(
                out=pc[:], lhsT=expT_sb[:, xc, yh * P:(yh + 1) * P],
                rhs=enc_sb[:, xc, :], start=(xc == 0), stop=(xc == XC - 1))
        c_sb = out_pool.tile([P, E], F32, tag="c_sb", name=f"c_sb{yh}")
        nc.vector.tensor_scalar_mul(
            out=c_sb[:], in0=pc[:], scalar1=recip_sb[:, yh:yh + 1])
        nc.sync.dma_start(out=c_d[yh * P:(yh + 1) * P, :], in_=c_sb[:])

    # ---- attention weights alpha[y,x] = expT[x,y]^T / denom[y] ----
    alpha_sb = consts.tile([P, YC, Tx], F32)
    for yh in range(YC):
        for xc in range(XC):
            pt2 = fin_psum.tile([P, E], F32, tag="pf", name="pt2")
            nc.tensor.transpose(
                out=pt2[:, :P], in_=expT_sb[:, xc, yh * P:(yh + 1) * P],
                identity=ident[:])
            nc.vector.tensor_scalar_mul(
                out=alpha_sb[:, yh, xc * P:(xc + 1) * P], in0=pt2[:, :P],
                scalar1=recip_sb[:, yh:yh + 1])
        nc.sync.dma_start(out=e_d[yh * P:(yh + 1) * P, :],
                          in_=alpha_sb[:, yh, :])


def _build():
    nc = bacc.Bacc("TRN2", target_bir_lowering=False, debug=False,
                   num_devices=NCORES)
    enc_d = nc.dram_tensor("enc", [Tx, E], F32, kind="ExternalInput").ap()
    dec_d = nc.dram_tensor("dec", [Ty, D], F32, kind="ExternalInput").ap()
    W_d = nc.dram_tensor("W", [E, E], F32, kind="ExternalInput").ap()
    U_d = nc.dram_tensor("U", [D, E], F32, kind="ExternalInput").ap()
    V_d = nc.dram_tensor("V", [E, 1], F32, kind="ExternalInput").ap()
    c_d = nc.dram_tensor("c_out", [Ty, E], F32, kind="ExternalOutput").ap()
    e_d = nc.dram_tensor("e_out", [Ty, Tx], F32, kind="ExternalOutput").ap()

    with tile.TileContext(nc) as tc:
        with ExitStack() as ctx:
            _build_body(tc, ctx, enc_d, dec_d, W_d, U_d, V_d, c_d, e_d)
    nc.compile()
    return nc


def _get_nc():
    global _NC
    if _NC is None:
        _NC = _build()
    return _NC


def kernel(encoder_out_seq, decoder_out_seq, W_a, U_a, V_a):
    enc = np.ascontiguousarray(np.asarray(encoder_out_seq, dtype=np.float32))
    dec = np.ascontiguousarray(np.asarray(decoder_out_seq, dtype=np.float32))
    W = np.ascontiguousarray(np.asarray(W_a, dtype=np.float32))
    U = np.ascontiguousarray(np.asarray(U_a, dtype=np.float32))
    V = np.ascontiguousarray(np.asarray(V_a, dtype=np.float32))

    nc = _get_nc()
    in_maps = [
        {"enc": enc[i], "dec": dec[i], "W": W, "U": U, "V": V}
        for i in range(NCORES)
    ]
    res = run_bass_kernel_spmd(nc, in_maps, list(range(NCORES)))
    global LAST_RESULTS
    LAST_RESULTS = res
    c = np.stack([res.results[i]["c_out"] for i in range(NCORES)])
    e = np.stack([res.results[i]["e_out"] for i in range(NCORES)])
    return c, e


# revision 20
# speedup vs baseline: 1.7554x; 1.0369x over previous
"""Bahdanau additive attention (vectorized) on TRN2 — Bass/Tile kernel.

Problem: nn_AttentionLayer_11055245820581
  e[b,y,x] = softmax_x( sum_e V[e] * tanh(Ws[b,x,e] + Uh[b,y,e]) )
  c[b,y,:] = sum_x e[b,y,x] * enc[b,x,:]
with Ws = enc @ W_a, Uh = dec @ U_a.

Sharding: data-parallel over batch B=8 across the 8 NeuronCores (one
batch element per core). Each core computes its batch's full attention.

Per-core dataflow (the tanh cube Ty*Tx*E = 16.7M elements dominates;
ACT's 1 elem/lane/cycle tanh is the ~110us floor, everything else is
arranged to stay below it):
  - broadcast-add WsT[e,x] + UhT[e,y] into fp16 slabs, split per block
    between DVE (tensor_scalar_add, per-partition scalar) and GPSIMD
    (one tensor_tensor with step-0 broadcast APs per 32-y slab) so
    neither engine exceeds ACT's per-block budget.
  - ACT: one big Tanh per (y-block, chunk) slab -> fp16 tanh slab.
  - PE: projection with tanh slab as fp16 stationary [128e, 128x] and
    V fp16 moving: e'^T lands as [x(partition), y] columns in PSUM
    (M=128 amortizes the per-matmul fixed cost; no PSUM evacuation).
  - softmax in the transposed layout: ACT Exp -> expT in SBUF; row sums
    over x via matmul with a ones vector -> denom[y]; DVE reciprocal;
    context matmul uses unnormalized expT and scales c rows by 1/denom;
    attention weights are PE-transposed back to [y, x] and scaled.
"""

import numpy as np
from contextlib import ExitStack

import concourse.bass as bass
import concourse.bacc as bacc
import concourse.tile as tile
from concourse import mybir
from concourse.bass_utils import run_bass_kernel_spmd

B, Tx, Ty, E, D = 8, 256, 256, 256, 256
P = 128
NCORES = 8
F32 = mybir.dt.float32
F16 = mybir.dt.float16
TANH = mybir.ActivationFunctionType.Tanh
EXP = mybir.ActivationFunctionType.Exp

G = 32           # y-block size of the main loop
NB = Ty // G     # 8 blocks
EC = E // P      # 2 e-chunks
XC = Tx // P     # 2 x-chunks
YC = Ty // P     # 2 y-halves
DC = D // P      # 2 d-chunks

_NC = None
LAST_RESULTS = None


def _bcast_add_ap(t, n_rep, n_inner):
    """AP reading a [P, n_inner] tile as [P, n_rep, n_inner] (repeat dim 1)."""
    return bass.AP(tensor=t.tensor, offset=t.offset,
                   ap=[t.ap[0], [0, n_rep], t.ap[1]])


def _bcast_inner_ap(t, col0, n_rep, n_inner):
    """AP reading tile columns [col0:col0+n_rep] as [P, n_rep, n_inner]
    (each column repeated n_inner times along the innermost dim)."""
    step = t.ap[1][0]
    return bass.AP(tensor=t.tensor, offset=t.offset + col0 * step,
                   ap=[t.ap[0], [step, n_rep], [0, n_inner]])


def _build_body(tc, ctx, enc_d, dec_d, W_d, U_d, V_d, c_d, e_d):
    nc = tc.nc
    from concourse.masks import make_identity

    consts = ctx.enter_context(tc.tile_pool(name="consts", bufs=1))
    add_pool = ctx.enter_context(tc.tile_pool(name="adds", bufs=4))
    tanh_pool = ctx.enter_context(tc.tile_pool(name="tanhs", bufs=4))
    out_pool = ctx.enter_context(tc.tile_pool(name="outs", bufs=2))
    setup_psum = ctx.enter_context(tc.tile_pool(name="psetup", bufs=2, space="PSUM"))
    e_psum = ctx.enter_context(tc.tile_pool(name="pe", bufs=1, space="PSUM"))
    fin_psum = ctx.enter_context(tc.tile_pool(name="pfin", bufs=1, space="PSUM"))
    piece_psum = ctx.enter_context(tc.tile_pool(name="ppiece", bufs=1, space="PSUM"))

    # ---- load inputs ----
    enc_sb = consts.tile([P, XC, E], F32)    # [x_in_chunk, (xc), e]
    dec_sb = consts.tile([P, YC, D], F32)
    W_sb = consts.tile([P, EC, E], F32)      # rows e_in
    U_sb = consts.tile([P, DC, E], F32)      # rows d
    V_sb = consts.tile([P, EC], F32)
    for i in range(XC):
        nc.sync.dma_start(out=enc_sb[:, i, :], in_=enc_d[i * P:(i + 1) * P, :])
    for i in range(YC):
        nc.sync.dma_start(out=dec_sb[:, i, :], in_=dec_d[i * P:(i + 1) * P, :])
    for i in range(EC):
        nc.sync.dma_start(out=W_sb[:, i, :], in_=W_d[i * P:(i + 1) * P, :])
    for i in range(DC):
        nc.sync.dma_start(out=U_sb[:, i, :], in_=U_d[i * P:(i + 1) * P, :])
    for i in range(EC):
        nc.sync.dma_start(out=V_sb[:, i:i + 1], in_=V_d[i * P:(i + 1) * P, :])

    ident = consts.tile([P, P], F32)
    make_identity(nc, ident)
    ident16 = consts.tile([P, P], F16)
    nc.vector.tensor_copy(ident16[:], ident[:])
    ones_sb = consts.tile([P, 1], F32)
    nc.vector.memset(ones_sb[:], 1.0)
    V16_sb = consts.tile([P, EC], F16)
    nc.vector.tensor_copy(V16_sb[:], V_sb[:])

    # ---- transpose enc, dec (PE transpose via identity) ----
    encT_sb = consts.tile([P, EC, Tx], F32)  # [e, (ec), x]
    decT_sb = consts.tile([P, DC, Ty], F32)  # [d, (dc), y]
    for src, srcC, dstT, dstC in ((enc_sb, XC, encT_sb, EC),
                                  (dec_sb, YC, decT_sb, DC)):
        for i in range(srcC):          # source partition chunk (x or y)
            for j in range(dstC):      # source free chunk (e or d)
                pt = setup_psum.tile([P, Tx], F32, tag="ps", name="pt")
                nc.tensor.transpose(
                    out=pt[:, :P], in_=src[:, i, j * P:(j + 1) * P],
                    identity=ident[:])
                nc.vector.tensor_copy(dstT[:, j, i * P:(i + 1) * P], pt[:, :P])

    # ---- WsT[e_out, x] = sum_ei W[ei, e_out] * encT[ei, x] ----
    # fp16 copies feed the DVE/GPSIMD adds; fp32 UhT feeds the DVE
    # per-partition scalar reads (TensorScalar requires fp32 scalars).
    WsT16_sb = consts.tile([P, EC, Tx], F16)
    UhT16_sb = consts.tile([P, EC, Ty], F16)
    UhT_sb = consts.tile([P, EC, Ty], F32)
    for co in range(EC):
        pw = setup_psum.tile([P, Tx], F32, tag="ps", name="pw")
        for ci in range(EC):
            nc.tensor.matmul(
                out=pw[:], lhsT=W_sb[:, ci, co * P:(co + 1) * P],
                rhs=encT_sb[:, ci, :], start=(ci == 0), stop=(ci == EC - 1))
        nc.vector.tensor_copy(WsT16_sb[:, co, :], pw[:])
    for co in range(EC):
        pu = setup_psum.tile([P, Ty], F32, tag="ps", name="pu")
        for ci in range(DC):
            nc.tensor.matmul(
                out=pu[:], lhsT=U_sb[:, ci, co * P:(co + 1) * P],
                rhs=decT_sb[:, ci, :], start=(ci == 0), stop=(ci == DC - 1))
        nc.vector.tensor_copy(UhT_sb[:, co, :], pu[:])
        nc.vector.tensor_copy(UhT16_sb[:, co, :], pu[:])

    # ---- main loop: tanh cube + V projection into e'^T ----
    # e'^T[x, y] accumulates into two [128, Ty] PSUM tiles (one per xc).
    eT_ps = [e_psum.tile([P, Ty], F32, tag=f"e{i}", name=f"eT_ps{i}")
             for i in range(XC)]
    for i in range(XC):
        nc.vector.memset(eT_ps[i][:], 0.0)

    # First PY y's of every (block, chunk) get their broadcast-add done on
    # the Tensor engine (identity matmuls of a step-0-broadcast W plus an
    # inner-broadcast U, accumulated bank-by-bank into one PSUM piece),
    # the rest on DVE via fp16 tensor_scalar. ACT tanh-reads the PSUM
    # piece in a single op.
    PY = 6
    for b in range(NB):
        y0 = b * G
        slabs = []
        for c in range(EC):
            tslab = tanh_pool.tile([P, G, Tx], F16, tag="tanh",
                                   name=f"tanh{b}_{c}")
            piece = piece_psum.tile([P, PY * Tx], F32, tag="piece",
                                    name=f"piece{b}_{c}")
            for s in range(PY // 2):
                sub = piece[:, 2 * Tx * s:2 * Tx * (s + 1)]
                nc.tensor.matmul(
                    out=sub,
                    lhsT=ident16[:],
                    rhs=_bcast_add_ap(WsT16_sb[:, c, :], 2, Tx),
                    start=True, stop=False)
                nc.tensor.matmul(
                    out=sub,
                    lhsT=ident16[:],
                    rhs=_bcast_inner_ap(UhT16_sb[:, c, :], y0 + 2 * s, 2, Tx),
                    start=False, stop=True)
            nc.scalar.activation(out=tslab[:, :PY, :], in_=piece[:],
                                 func=TANH)
            aslab = add_pool.tile([P, G - PY, Tx], F16, tag="add",
                                  name=f"add{b}_{c}")
            for j in range(G - PY):
                nc.vector.tensor_scalar_add(
                    out=aslab[:, j, :], in0=WsT16_sb[:, c, :],
                    scalar1=UhT_sb[:, c, y0 + PY + j:y0 + PY + j + 1])
            nc.scalar.activation(out=tslab[:, PY:, :], in_=aslab[:],
                                 func=TANH)
            slabs.append(tslab)
        for j in range(G):
            for xc in range(XC):
                for c in range(EC):
                    nc.tensor.matmul(
                        out=eT_ps[xc][:, y0 + j:y0 + j + 1],
                        lhsT=slabs[c][:, j, xc * P:(xc + 1) * P],
                        rhs=V16_sb[:, c:c + 1],
                        start=False, stop=False,
                        skip_group_check=True)

    # ---- softmax over x (partition dim of e'^T) ----
    expT_sb = consts.tile([P, XC, Ty], F32)  # [x, (xc), y]
    for xc in range(XC):
        nc.scalar.activation(out=expT_sb[:, xc, :], in_=eT_ps[xc][:], func=EXP)
    den_ps = fin_psum.tile([P, YC], F32, tag="pf", name="den_ps")
    nc.vector.memset(den_ps[:], 0.0)
    for yh in range(YC):
        for xc in range(XC):
            nc.tensor.matmul(
                out=den_ps[:, yh:yh + 1],
                lhsT=expT_sb[:, xc, yh * P:(yh + 1) * P],
                rhs=ones_sb[:],
                start=False, stop=False, skip_group_check=True)
    recip_sb = consts.tile([P, YC], F32)
    nc.vector.reciprocal(recip_sb[:], den_ps[:])

    # ---- context c[y,:] = (sum_x expT[x,y] * enc[x,:]) / denom[y] ----
    for yh in range(YC):
        pc = fin_psum.tile([P, E], F32, tag="pf", name=f"pc{yh}")
        for xc in range(XC):
            nc.tensor.matmul(
                out=pc[:], lhsT=expT_sb[:, xc, yh * P:(yh + 1) * P],
                rhs=enc_sb[:, xc, :], start=(xc == 0), stop=(xc == XC - 1))
        c_sb = out_pool.tile([P, E], F32, tag="c_sb", name=f"c_sb{yh}")
        nc.vector.tensor_scalar_mul(
            out=c_sb[:], in0=pc[:], scalar1=recip_sb[:, yh:yh + 1])
        nc.sync.dma_start(out=c_d[yh * P:(yh + 1) * P, :], in_=c_sb[:])

    # ---- attention weights alpha[y,x] = expT[x,y]^T / denom[y] ----
    alpha_sb = consts.tile([P, YC, Tx], F32)
    for yh in range(YC):
        for xc in range(XC):
            pt2 = fin_psum.tile([P, E], F32, tag="pf", name="pt2")
            nc.tensor.transpose(
                out=pt2[:, :P], in_=expT_sb[:, xc, yh * P:(yh + 1) * P],
                identity=ident[:])
            nc.vector.tensor_scalar_mul(
                out=alpha_sb[:, yh, xc * P:(xc + 1) * P], in0=pt2[:, :P],
                scalar1=recip_sb[:, yh:yh + 1])
        nc.sync.dma_start(out=e_d[yh * P:(yh + 1) * P, :],
                          in_=alpha_sb[:, yh, :])


def _build():
    nc = bacc.Bacc("TRN2", target_bir_lowering=False, debug=False,
                   num_devices=NCORES)
    enc_d = nc.dram_tensor("enc", [Tx, E], F32, kind="ExternalInput").ap()
    dec_d = nc.dram_tensor("dec", [Ty, D], F32, kind="ExternalInput").ap()
    W_d = nc.dram_tensor("W", [E, E], F32, kind="ExternalInput").ap()
    U_d = nc.dram_tensor("U", [D, E], F32, kind="ExternalInput").ap()
    V_d = nc.dram_tensor("V", [E, 1], F32, kind="ExternalInput").ap()
    c_d = nc.dram_tensor("c_out", [Ty, E], F32, kind="ExternalOutput").ap()
    e_d = nc.dram_tensor("e_out", [Ty, Tx], F32, kind="ExternalOutput").ap()

    with tile.TileContext(nc) as tc:
        with ExitStack() as ctx:
            _build_body(tc, ctx, enc_d, dec_d, W_d, U_d, V_d, c_d, e_d)
    nc.compile()
    return nc


def _get_nc():
    global _NC
    if _NC is None:
        _NC = _build()
    return _NC


def kernel(encoder_out_seq, decoder_out_seq, W_a, U_a, V_a):
    enc = np.ascontiguousarray(np.asarray(encoder_out_seq, dtype=np.float32))
    dec = np.ascontiguousarray(np.asarray(decoder_out_seq, dtype=np.float32))
    W = np.ascontiguousarray(np.asarray(W_a, dtype=np.float32))
    U = np.ascontiguousarray(np.asarray(U_a, dtype=np.float32))
    V = np.ascontiguousarray(np.asarray(V_a, dtype=np.float32))

    nc = _get_nc()
    in_maps = [
        {"enc": enc[i], "dec": dec[i], "W": W, "U": U, "V": V}
        for i in range(NCORES)
    ]
    res = run_bass_kernel_spmd(nc, in_maps, list(range(NCORES)))
    global LAST_RESULTS
    LAST_RESULTS = res
    c = np.stack([res.results[i]["c_out"] for i in range(NCORES)])
    e = np.stack([res.results[i]["e_out"] for i in range(NCORES)])
    return c, e
